# revision 1
# baseline (speedup 1.0000x reference)
import jax
import jax.numpy as jnp
import numpy as np
from functools import partial

# nn_GCN_17008070492360: N=100000, K=16, F=128, H1=64, H2=32, C=10
# Data-parallel over nodes across 8 NeuronCores; W1/W2/Wc replicated.
N, K, F = 100000, 16, 128
EPS = 1e-12
NCORES = 8


def _l2norm(a):
    n = jnp.linalg.norm(a, axis=-1, keepdims=True)
    return a / jnp.maximum(n, EPS)


@partial(jax.pmap, axis_name="i", in_axes=(0, 0, None, None, None))
def _shard_fn(x, neighbor, W1, W2, Wc):
    x1 = _l2norm(x) @ W1.T                      # [n, H1]
    nb1 = _l2norm(neighbor) @ W1.T              # [n, K, H1]
    agg = jax.nn.relu(nb1.sum(axis=1))          # self path (unused downstream)
    nbs = jax.nn.relu(x1[:, None, :] + nb1)     # [n, K, H1]
    x2 = nbs.sum(axis=1) @ W2.T                 # sum_k (relu(...) @ W2.T)
    del agg
    return jax.nn.relu(x2) @ Wc.T               # [n, C]


def kernel(x, neighbor, W1, W2, Wc):
    x = np.asarray(x, dtype=np.float32)
    neighbor = np.asarray(neighbor, dtype=np.float32)
    n_per = N // NCORES
    xs = x.reshape(NCORES, n_per, F)
    nbs = neighbor.reshape(NCORES, n_per, K, F)
    out = _shard_fn(xs, nbs, jnp.asarray(W1), jnp.asarray(W2), jnp.asarray(Wc))
    return np.asarray(out).reshape(N, -1).astype(np.float32)



# revision 18
# speedup vs baseline: 20665.7064x; 20665.7064x over previous
"""GCN message-passing kernel for Trainium2 (Bass/Tile), data-parallel over
nodes across 8 NeuronCores.

Math per node i (see reference):
  xn  = l2norm(x[i]);  zn_k = l2norm(neighbor[i,k])
  r_k = relu((xn + zn_k) @ W1.T)           # == relu(x1n + nb1n_k)
  s   = sum_k r_k
  out = relu(s @ W2.T) @ Wc.T

Host prep: inputs are sharded by node and pre-swizzled to partition-major
[128, tiles, F] so every SWDGE DMA is 128 large contiguous descriptors.

Device pipeline per 128-row tile (1 tile == 8 nodes x 16 neighbors), all
compute in bf16 with f32 PSUM matmul accumulation:
  - SWDGE DMA loads cast f32->bf16 on the fly (~6MB chunks)
  - ssq per row via scalar_tensor_tensor accum (split DVE/GPSIMD)
  - inv-norm via ACT sqrt + DVE reciprocal (batched per load)
  - row scale via tensor_scalar_mul (per-partition scalar)
  - PE bf16 transpose (is_transpose), 8 tiles per PSUM bank
  - one batched bf16 evac PSUM->SBUF per 8 tiles (alternating DVE/ACT)
  - W1 matmul (data stationary) + broadcast-add of x1n via a shifted
    selector matmul into the same PSUM accumulation slice
  - ACT relu evac batched over 8 tiles per PSUM bank
  - transposed k-sum (r-tile stationary vs [128,8] selector) accumulating
    sT [H1, 512 nodes] per PSUM bank
  - layers 2/3 per 512-node chunk (W2 / relu / transpose / Wc)

The 4 leftover nodes per core (12500 % 8) are computed on host in numpy.
"""

import os
from contextlib import ExitStack

import numpy as np
import ml_dtypes

import concourse.bass as bass
import concourse.bacc as bacc
import concourse.tile as tile
import concourse.mybir as mybir
from concourse import bass_utils

F32 = mybir.dt.float32
BF16 = mybir.dt.bfloat16
AL = mybir.AluOpType
AF = mybir.ActivationFunctionType

N, K, F = 100000, 16, 128
H1, H2, C = 64, 32, 10
NCORES = 8
NLOC = N // NCORES            # 12500 nodes per core
GRP = 8                       # nodes per tile (8*16 = 128 rows)
NG = NLOC // GRP              # 1562 full groups (tiles) per core
NDEV = NG * GRP               # 12496 device-computed nodes
NHOST = NLOC - NDEV           # 4 host-computed nodes per core
XTILES = (NLOC + 127) // 128  # 98 x tiles (rows padded with ones)
CPAD = 16                     # Wc output columns padded
SCH = 64                      # groups per sT chunk (512 nodes per PSUM bank)

# engine-split knobs, tuned from traces
DVE_EVAC = 0                  # of 8 evac batches go to DVE, rest ACT
DVE_SCALE = 3                 # of 8 scale ops per batch on DVE, rest GPSIMD

TILES_PER_LOAD = 96           # neighbor tiles per SWDGE DMA (6MB f32 src)


def build_nc(ng=NG, xtiles=XTILES):
    """Build the Bass program. ng = neighbor groups (tiles); xtiles = x tiles."""
    nc = bacc.Bacc("TRN2", target_bir_lowering=False)

    x_d = nc.dram_tensor("xp", [128, xtiles, F], F32, kind="ExternalInput").ap()
    nb_d = nc.dram_tensor("nb", [128, ng, F], F32, kind="ExternalInput").ap()
    w1t_d = nc.dram_tensor("w1t", [F, H1], BF16, kind="ExternalInput").ap()
    w2t_d = nc.dram_tensor("w2t", [H1, H2], BF16, kind="ExternalInput").ap()
    wct_d = nc.dram_tensor("wct", [H2, CPAD], BF16, kind="ExternalInput").ap()
    idn_d = nc.dram_tensor("idn", [128, 128], BF16, kind="ExternalInput").ap()
    s16_d = nc.dram_tensor("s16", [128, GRP], BF16, kind="ExternalInput").ap()
    sBg_d = nc.dram_tensor("sBg", [128, 16, 128], BF16, kind="ExternalInput").ap()
    out_d = nc.dram_tensor("out", [ng * GRP, C], F32, kind="ExternalOutput").ap()

    with tile.TileContext(nc) as tc, ExitStack() as ctx:
        _body(ctx, tc, ng, xtiles, x_d, nb_d, w1t_d, w2t_d, wct_d, idn_d,
              s16_d, sBg_d, out_d)
    nc.compile()
    return nc


def _body(ctx, tc, ng, xtiles, x_d, nb_d, w1t_d, w2t_d, wct_d, idn_d, s16_d,
          sBg_d, out_d):
    nc = tc.nc

    singles = ctx.enter_context(tc.tile_pool(name="singles", bufs=1))
    loads = ctx.enter_context(tc.tile_pool(name="loads", bufs=3))
    evacs = ctx.enter_context(tc.tile_pool(name="evacs", bufs=4))
    scratch = ctx.enter_context(tc.tile_pool(name="scratch", bufs=8))
    relus = ctx.enter_context(tc.tile_pool(name="relus", bufs=4))
    p2 = ctx.enter_context(tc.tile_pool(name="p2", bufs=3))
    # PSUM: 8 banks total; every tile rounds up to one 2KB bank.
    tpsum = ctx.enter_context(tc.tile_pool(name="tpsum", bufs=2, space="PSUM"))
    y1psum = ctx.enter_context(tc.tile_pool(name="y1psum", bufs=2, space="PSUM"))
    spsum = ctx.enter_context(tc.tile_pool(name="spsum", bufs=2, space="PSUM"))
    p2psum = ctx.enter_context(tc.tile_pool(name="p2psum", bufs=2, space="PSUM"))

    # constants
    w1t = singles.tile([F, H1], BF16)
    nc.sync.dma_start(w1t, w1t_d)
    w2t = singles.tile([H1, H2], BF16)
    nc.sync.dma_start(w2t, w2t_d)
    wct = singles.tile([H2, CPAD], BF16)
    nc.sync.dma_start(wct, wct_d)
    idn = singles.tile([128, 128], BF16)
    nc.sync.dma_start(idn, idn_d)
    s16 = singles.tile([128, GRP], BF16)
    nc.sync.dma_start(s16, s16_d)
    sBg = singles.tile([128, 16, 128], BF16)
    nc.sync.dma_start(sBg, sBg_d)

    # persistent buffers
    zx = singles.tile([128, xtiles, F], BF16)      # x rows, scaled in place
    x1n = singles.tile([128, xtiles, H1], BF16)    # x1n rows
    xnorm = singles.tile([128, xtiles], F32)
    xinv = singles.tile([128, xtiles], F32)
    nnorm = singles.tile([128, ng], F32)
    ninv = singles.tile([128, ng], F32)

    def scale_tile(ztile, inv_col, on_dve):
        if on_dve:
            nc.vector.tensor_scalar_mul(ztile, ztile, inv_col)
        else:
            # Pool runs tensor_tensor via the standard Q7 library; broadcast
            # the per-row inv-norm along the free dim.
            nc.gpsimd.tensor_tensor(ztile, ztile,
                                    inv_col.to_broadcast((128, F)), AL.mult)

    def transpose_batch(stiles, evac_dve):
        """Transpose `len(stiles)` bf16 [128,128] tiles through one PSUM bank
        and evacuate with a single batched op. Returns SBUF [128, 8, 128]."""
        bl = len(stiles)
        tp = tpsum.tile([128, GRP, 128], BF16, tag="tp")
        for j, st in enumerate(stiles):
            nc.tensor.transpose(tp[:, j, :], st, idn)
        zT = evacs.tile([128, GRP, 128], BF16, tag="ev")
        if evac_dve:
            nc.vector.tensor_copy(zT[:, :bl, :], tp[:, :bl, :])
        else:
            nc.scalar.copy(zT[:, :bl, :], tp[:, :bl, :])
        return zT

    # ---- x pipeline: x1n[i] = l2norm(x[i]) @ W1.T ----
    nc.gpsimd.dma_start(out=zx, in_=x_d)
    # touch ops absorb the DMA-sem wait (STT structs fit only one sync wait)
    tchV = scratch.tile([128, 1], BF16, tag="touchV")
    nc.vector.tensor_copy(tchV, zx[:, 0, 0:1])
    for t in range(xtiles):
        scr = scratch.tile([128, F], BF16, tag="scrV")
        nc.vector.scalar_tensor_tensor(
            out=scr, in0=zx[:, t, :], scalar=1.0, in1=zx[:, t, :],
            op0=AL.bypass, op1=AL.mult, accum_out=xnorm[:, t : t + 1])
    nc.scalar.activation(out=xnorm, in_=xnorm, func=AF.Sqrt)
    nc.vector.reciprocal(out=xinv, in_=xnorm)
    for b0 in range(0, xtiles, 8):
        bl = min(8, xtiles - b0)
        for t in range(b0, b0 + bl):
            scale_tile(zx[:, t, :], xinv[:, t : t + 1], t % 8 < DVE_SCALE)
        zT = transpose_batch([zx[:, t, :] for t in range(b0, b0 + bl)],
                             (b0 // 8) % 8 < DVE_EVAC)
        y1x = y1psum.tile([128, GRP, H1], F32, tag="y1")
        for j in range(bl):
            nc.tensor.matmul(y1x[:, j, :], lhsT=zT[:, j, :], rhs=w1t,
                             start=True, stop=True)
        nc.vector.tensor_copy(x1n[:, b0 : b0 + bl, :], y1x[:, :bl, :])

    # ---- neighbor pipeline ----
    nloads = (ng + TILES_PER_LOAD - 1) // TILES_PER_LOAD
    sT_ps = None
    s_count = 0
    chunk = 0
    for ld in range(nloads):
        t0 = ld * TILES_PER_LOAD
        tl = min(TILES_PER_LOAD, ng - t0)
        znb = loads.tile([128, TILES_PER_LOAD, F], BF16, tag="znb")
        nc.gpsimd.dma_start(out=znb[:, :tl, :], in_=nb_d[:, t0 : t0 + tl, :])
        tchV = scratch.tile([128, 1], BF16, tag="touchV")
        nc.vector.tensor_copy(tchV, znb[:, 0, 0:1])
        tchP = scratch.tile([128, 1], BF16, tag="touchP")
        nc.gpsimd.tensor_copy(tchP, znb[:, 0, 0:1])
        # ssq + inv-norm in 16-tile slices (fine-grained for pipelining)
        for s0 in range(0, tl, 16):
            sl = min(16, tl - s0)
            for t in range(s0, s0 + sl):
                g = t0 + t
                scr = scratch.tile([128, F], BF16, tag="scrV")
                nc.vector.scalar_tensor_tensor(
                    out=scr, in0=znb[:, t, :], scalar=1.0, in1=znb[:, t, :],
                    op0=AL.bypass, op1=AL.mult,
                    accum_out=nnorm[:, g : g + 1])
            nc.scalar.activation(out=nnorm[:, t0 + s0 : t0 + s0 + sl],
                                 in_=nnorm[:, t0 + s0 : t0 + s0 + sl],
                                 func=AF.Sqrt)
            nc.vector.reciprocal(out=ninv[:, t0 + s0 : t0 + s0 + sl],
                                 in_=nnorm[:, t0 + s0 : t0 + s0 + sl])

        # process tiles in batches of 8 (one y1 PSUM bank per batch)
        for b0 in range(0, tl, 8):
            bl = min(8, tl - b0)
            for t in range(b0, b0 + bl):
                g = t0 + t
                scale_tile(znb[:, t, :], ninv[:, g : g + 1], t % 8 < DVE_SCALE)
            zT = transpose_batch([znb[:, t, :] for t in range(b0, b0 + bl)],
                                 (b0 // 8) % 8 < DVE_EVAC)
            y1 = y1psum.tile([128, GRP, H1], F32, tag="y1")
            r_sb = relus.tile([128, GRP, H1], BF16, tag="r")
            for t in range(b0, b0 + bl):
                g = t0 + t
                j = t - b0
                nc.tensor.matmul(y1[:, j, :], lhsT=zT[:, j, :], rhs=w1t,
                                 start=True, stop=False)
                nc.tensor.matmul(y1[:, j, :], lhsT=sBg[:, g % 16, :],
                                 rhs=x1n[:, g // 16, :], start=False, stop=True)
            nc.scalar.activation(out=r_sb[:, :bl, :], in_=y1[:, :bl, :],
                                 func=AF.Relu)
            for t in range(b0, b0 + bl):
                g = t0 + t
                if sT_ps is None:
                    sT_ps = spsum.tile([H1, SCH * GRP], F32, tag="sT")
                    s_count = 0
                c0 = (g % SCH) * GRP
                nc.tensor.matmul(sT_ps[:, c0 : c0 + GRP],
                                 lhsT=r_sb[:, t - b0, :], rhs=s16,
                                 start=True, stop=True)
                s_count += 1
                if s_count == SCH or g == ng - 1:
                    _phase2(nc, p2, p2psum, idn, w2t, wct, out_d,
                            sT_ps, chunk, s_count * GRP)
                    sT_ps = None
                    chunk += 1


def _phase2(nc, p2, p2psum, idn, w2t, wct, out_d, sT_ps, chunk, nn):
    """Layers 2+3 for one chunk of up to 512 nodes held in sT_ps [H1, nn]."""
    nq = (nn + 127) // 128
    sT = p2.tile([H1, SCH * GRP], BF16, tag="sT_sb")
    nc.vector.tensor_copy(sT[:, :nn], sT_ps[:, :nn])
    x2p = p2psum.tile([128, 4, H2], F32, tag="p2p")
    for q in range(nq):
        pn = min(128, nn - q * 128)
        nc.tensor.matmul(x2p[:pn, q, :], lhsT=sT[:, q * 128 : q * 128 + pn],
                         rhs=w2t, start=True, stop=True)
    x2r = p2.tile([128, 4, H2], BF16, tag="x2r")
    nfull = nn // 128
    pn_last = nn - nfull * 128
    if nfull:
        nc.scalar.activation(out=x2r[:, :nfull, :], in_=x2p[:, :nfull, :],
                             func=AF.Relu)
    if pn_last:
        nc.scalar.activation(out=x2r[:pn_last, nfull, :],
                             in_=x2p[:pn_last, nfull, :], func=AF.Relu)
    pt2 = p2psum.tile([H2, 4, 128], F32, tag="p2p")
    for q in range(nq):
        pn = min(128, nn - q * 128)
        nc.tensor.matmul(pt2[:, q, :pn], lhsT=x2r[:pn, q, :],
                         rhs=idn[:pn, :pn], start=True, stop=True)
    x2rT = p2.tile([H2, 4, 128], BF16, tag="x2rT")
    if nfull:
        nc.vector.tensor_copy(x2rT[:, :nfull, :], pt2[:, :nfull, :])
    if pn_last:
        nc.vector.tensor_copy(x2rT[:, nfull, :pn_last], pt2[:, nfull, :pn_last])
    op = p2psum.tile([128, 4, CPAD], F32, tag="p2p")
    for q in range(nq):
        pn = min(128, nn - q * 128)
        nc.tensor.matmul(op[:pn, q, :], lhsT=x2rT[:, q, :pn], rhs=wct,
                         start=True, stop=True)
    o_sb = p2.tile([128, 4, C], F32, tag="o_sb")
    if nfull:
        nc.scalar.copy(o_sb[:, :nfull, :], op[:, :nfull, :C])
    if pn_last:
        nc.scalar.copy(o_sb[:pn_last, nfull, :], op[:pn_last, nfull, :C])
    base = chunk * SCH * GRP
    for q in range(nq):
        pn = min(128, nn - q * 128)
        nc.sync.dma_start(out_d[base + q * 128 : base + q * 128 + pn, :],
                          o_sb[:pn, q, :])


# ---------------- host side ----------------

_NC_CACHE = {}


def _get_nc():
    if "nc" not in _NC_CACHE:
        _NC_CACHE["nc"] = build_nc()
    return _NC_CACHE["nc"]


def host_consts():
    bf = ml_dtypes.bfloat16
    idn = np.eye(128, dtype=bf)
    s16 = np.zeros((128, GRP), dtype=bf)
    for r in range(128):
        s16[r, r // K] = 1
    # sBg[j, v, r] = 1 iff j == 8*v + r//16 (x1n row selector, shifted)
    sBg = np.zeros((128, 16, 128), dtype=bf)
    for v in range(16):
        for r in range(128):
            sBg[8 * v + r // K, v, r] = 1
    return idn, s16, sBg


def host_weights(W1, W2, Wc):
    bf = ml_dtypes.bfloat16
    w1t = np.ascontiguousarray(W1.T).astype(bf)            # [F, H1]
    w2t = np.ascontiguousarray(W2.T).astype(bf)            # [H1, H2]
    wct = np.zeros((H2, CPAD), dtype=bf)
    wct[:, :C] = np.ascontiguousarray(Wc.T).astype(bf)
    return w1t, w2t, wct


def host_ref(x, neighbor, W1, W2, Wc):
    """Exact f32 reference for the host-computed remainder nodes."""
    eps = 1e-12
    xn = x / np.maximum(np.linalg.norm(x, axis=-1, keepdims=True), eps)
    nb = neighbor / np.maximum(
        np.linalg.norm(neighbor, axis=-1, keepdims=True), eps)
    x1 = xn @ W1.T
    nb1 = nb @ W1.T
    nbs = np.maximum(x1[:, None, :] + nb1, 0.0)
    s = nbs.sum(axis=1)
    return np.maximum(s @ W2.T, 0.0) @ Wc.T


def make_in_maps(x, neighbor, W1, W2, Wc):
    w1t, w2t, wct = host_weights(W1, W2, Wc)
    idn, s16, sBg = host_consts()
    in_maps = []
    for c in range(NCORES):
        xs = x[c * NLOC : (c + 1) * NLOC]
        xp = np.ones((XTILES * 128, F), dtype=np.float32)
        xp[:NLOC] = xs
        # partition-major swizzle: [T,128,F] -> [128,T,F] (contiguous per
        # partition, so each SWDGE DMA is 128 big contiguous descriptors)
        xp = np.ascontiguousarray(
            xp.reshape(XTILES, 128, F).transpose(1, 0, 2))
        nbs = neighbor[c * NLOC : c * NLOC + NDEV].reshape(NG, 128, F)
        nbs = np.ascontiguousarray(nbs.transpose(1, 0, 2))
        in_maps.append({
            "xp": xp, "nb": nbs, "w1t": w1t, "w2t": w2t, "wct": wct,
            "idn": idn, "s16": s16, "sBg": sBg,
        })
    return in_maps


def kernel(x, neighbor, W1, W2, Wc):
    x = np.ascontiguousarray(np.asarray(x, dtype=np.float32))
    neighbor = np.ascontiguousarray(np.asarray(neighbor, dtype=np.float32))
    W1 = np.asarray(W1, dtype=np.float32)
    W2 = np.asarray(W2, dtype=np.float32)
    Wc = np.asarray(Wc, dtype=np.float32)

    nc = _get_nc()
    in_maps = make_in_maps(x, neighbor, W1, W2, Wc)
    trace = os.environ.get("GCN_TRACE") == "1"
    res = bass_utils.run_bass_kernel_spmd(nc, in_maps,
                                          core_ids=list(range(NCORES)),
                                          trace=trace)
    _NC_CACHE["last_result"] = res
    out = np.empty((N, C), dtype=np.float32)
    for c in range(NCORES):
        out[c * NLOC : c * NLOC + NDEV] = res.results[c]["out"]
        lo = c * NLOC + NDEV
        out[lo : lo + NHOST] = host_ref(
            x[lo : lo + NHOST], neighbor[lo : lo + NHOST], W1, W2, Wc)
    return out
